# revision 13
# baseline (speedup 1.0000x reference)
"""Multi-head causal attention (B=4, S=2048, D=1024, H=16, dk=dv=64) on 8
Trainium2 NeuronCores.

Sharding: batch (4) x head-group (2) -> 8 cores. Each core computes, for its
batch b and its 8 heads, the partial output (concat_g @ WO_g)^T in [D, S]
layout. Host sums the two head-group partials per batch, transposes, adds bO.

Per-core kernel (single SPMD Bass program, per-core data):
  P1: Q^T, K^T = (X @ WQg)^T, (X @ WKg)^T  as [512, 2048]  (j on partitions)
  P2: Vbar[s, h, 0:64] = (X @ WVg)[s, 512], Vbar[s, h, 64] = 1.0 (ones col)
  P3 (software-pipelined, skew 2, pair-grouped): per iteration
      (q-block 512, head pair, k-chunk 128):
      front(i): scoresT[k, q] for both heads of the pair into one wide
      [128,1024] PSUM tile (K=64 row-quadrants, adjacent so the PE co-issues
      them), one wide exp(0.125*scores) on the scalar engine, tri-mask of the
      diagonal-crossing cols on DVE.
      back(i-2): fv_p[0:65, q] += Vbar-chunk.T @ at_p  (row 64 = denom).
      Iterations are emitted in PAIRS ([sc(g) sc(g+1)] then [fv(g-2)
      fv(g-3)]) and every tensor matmul carries a nosync edge to the
      previous one, pinning the engine order: score and fv matmuls never
      alternate (PE tile-reconfig flush), and the exp latency is hidden by
      two iterations of skew.
      hp end: denom rows -> reciprocal_approx_fast -> gpsimd broadcast ->
      fused scale+cast into CT (bf16).
  P4 (lagged into the next q-block's scores): OUTT[m, q] = sum_c
      WOg-chunk.T @ CT-chunk, staged out via scalar/vector copies + DMA.

dtypes: inputs bf16; scores/fv accumulate f32 in PSUM; attn tiles bf16.
"""

import sys

sys.path.insert(0, "/opt/trn_rl_repo")

import numpy as np
import ml_dtypes

import concourse.bass as bass
from concourse import bacc
import concourse.tile as tile
from concourse import mybir
from concourse.bass_utils import run_bass_kernel_spmd

F32 = mybir.dt.float32
BF16 = mybir.dt.bfloat16
EXP = mybir.ActivationFunctionType.Exp
COPY = mybir.ActivationFunctionType.Copy
ADT = BF16   # dtype for Q/K/V/attn tiles (attention matmuls)

S = 2048          # sequence length
D = 1024          # model dim
HG = 8            # heads per core
DK = 64           # head dim
JG = HG * DK      # 512 = projected dim per core
CK = D // 128     # 8 contraction chunks for projections
NJT = JG // 128   # 4 j-tiles (head pairs)
NSB = S // 512    # 4 s-blocks of 512
NST = S // 128    # 16 s-tiles of 128
NQB = S // 512    # 4 q-blocks of 512

SKEW = 2          # fv trails scores by this many (qb,hp,kc) iterations
PLAG = 6          # out-projection trails the end of its q-block by this many

_NC_CACHE = {}


def build_nc(salt=""):
    nc = bacc.Bacc("TRN2", target_bir_lowering=False)

    XT_d = nc.declare_dram_parameter("XT", [D, S], BF16, isOutput=False)
    WQ_d = nc.declare_dram_parameter("WQ", [D, JG], BF16, isOutput=False)
    WK_d = nc.declare_dram_parameter("WK", [D, JG], BF16, isOutput=False)
    WV_d = nc.declare_dram_parameter("WV", [D, JG], BF16, isOutput=False)
    WO_d = nc.declare_dram_parameter("WO", [JG, D], BF16, isOutput=False)
    OUT_d = nc.declare_dram_parameter("OUTT", [D, S], F32, isOutput=True)

    # tril-inclusive mask in (k, q) orientation: mask[kr, qr] = 1 iff qr >= kr
    tri_np = (np.arange(128)[None, :] >= np.arange(128)[:, None]).astype(
        mybir.dt.np(ADT))
    TRI_d = nc.inline_tensor(tri_np, name="trimask" + salt)

    # Pin the tensor-engine order to emission order (see module docstring).
    import bass_rust as _br
    prev_mm = [None]

    def tmm(out, lhsT, rhs, **kw):
        mm = nc.tensor.matmul(out, lhsT, rhs, **kw)
        if prev_mm[0] is not None:
            deps = _br.InstructionNameOrderedSet()
            deps.add(prev_mm[0])
            mm.ins.add_nosync_dependencies_from(deps)
        prev_mm[0] = mm.ins.name
        return mm

    with tile.TileContext(nc) as tc:
        with (
            tc.tile_pool(name="persist", bufs=1) as pp,
            tc.tile_pool(name="psum", bufs=1, space="PSUM") as ps,
        ):
            tri = pp.tile([128, 128], ADT, tag="tri", name="tri")
            nc.sync.dma_start(tri[:], TRI_d[:])

            QT = [pp.tile([128, S], ADT, tag=f"qt{j}", name=f"qt{j}")
                  for j in range(NJT)]
            KT = [pp.tile([128, S], ADT, tag=f"kt{j}", name=f"kt{j}")
                  for j in range(NJT)]
            VB = [pp.tile([128, HG, DK + 1], ADT, tag=f"vb{s}", name=f"vb{s}")
                  for s in range(NST)]

            with tc.tile_pool(name="stage", bufs=1) as sp:
                XT = [sp.tile([128, S], BF16, tag=f"xt{c}", name=f"xt{c}")
                      for c in range(CK)]
                WQ = [sp.tile([128, JG], BF16, tag=f"wq{c}", name=f"wq{c}")
                      for c in range(CK)]
                WK = [sp.tile([128, JG], BF16, tag=f"wk{c}", name=f"wk{c}")
                      for c in range(CK)]
                WV = [sp.tile([128, JG], BF16, tag=f"wv{c}", name=f"wv{c}")
                      for c in range(CK)]
                # DMA in consumption order: P1-Q needs XT+WQ first.
                for c in range(CK):
                    nc.sync.dma_start(XT[c][:], XT_d[128 * c : 128 * (c + 1), :])
                    nc.sync.dma_start(WQ[c][:], WQ_d[128 * c : 128 * (c + 1), :])
                for c in range(CK):
                    nc.sync.dma_start(WK[c][:], WK_d[128 * c : 128 * (c + 1), :])
                for c in range(CK):
                    nc.sync.dma_start(WV[c][:], WV_d[128 * c : 128 * (c + 1), :])

                # ---- P1: Q^T and K^T projections ([j, s] layout) ----
                # Wide PSUM tiles: two 512-col s-blocks per tile, one wide
                # PSUM->SBUF cast per pair.
                for W, OT in ((WQ, QT), (WK, KT)):
                    for jt in range(NJT):
                        for sb in (0, 2):
                            pq = ps.tile([128, 1024], F32, tag="scw",
                                         name="scw", bufs=2)
                            for half in (0, 1):
                                for c in range(CK):
                                    tmm(
                                        pq[:, 512 * half : 512 * (half + 1)],
                                        W[c][:, 128 * jt : 128 * (jt + 1)],
                                        XT[c][:, 512 * (sb + half)
                                              : 512 * (sb + half + 1)],
                                        start=(c == 0),
                                        stop=(c == CK - 1),
                                    )
                            nc.vector.tensor_copy(
                                OT[jt][:, 512 * sb : 512 * (sb + 2)], pq[:]
                            )

                # ---- P2: V projection, seq-major, with ones column ----
                for st2 in range(0, NST, 2):
                    pv = ps.tile([128, 1024], F32, tag="scw", name="scw",
                                 bufs=2)
                    for half in (0, 1):
                        st = st2 + half
                        for c in range(CK):
                            tmm(
                                pv[:, 512 * half : 512 * (half + 1)],
                                XT[c][:, 128 * st : 128 * (st + 1)],
                                WV[c][:],
                                start=(c == 0),
                                stop=(c == CK - 1),
                            )
                    for half in (0, 1):
                        st = st2 + half
                        nc.vector.tensor_copy(
                            VB[st][:, :, 0:DK],
                            pv[:, 512 * half : 512 * (half + 1)].rearrange(
                                "p (h d) -> p h d", h=HG),
                        )
                        nc.gpsimd.memset(VB[st][:, :, DK : DK + 1], 1.0)

            with tc.tile_pool(name="late", bufs=1) as lp:
                WO = [lp.tile([128, D], BF16, tag=f"wo{c}", name=f"wo{c}")
                      for c in range(NJT)]
                for c in range(NJT):
                    nc.sync.dma_start(
                        WO[c][:], WO_d[128 * c : 128 * (c + 1), :]
                    )
                CT = [
                    [lp.tile([128, 512], BF16, tag=f"ct{qb}_{hp}",
                             name=f"ct{qb}_{hp}") for hp in range(NJT)]
                    for qb in range(NQB)
                ]

                # ---- P3: software-pipelined attention ----
                iters = [(qb, hp, kc)
                         for qb in range(NQB)
                         for hp in range(NJT)
                         for kc in range(4 * qb + 4)]
                n_it = len(iters)
                at_tiles = {}   # i -> wide at tile
                fv_tiles = {}   # (qb, hp) -> (fv_p0, fv_p1)
                pending_p4 = [] # (emit_step, qb)

                def emit_front(i):
                    qb, hp, kc = iters[i]
                    o = max(0, 128 * kc - 512 * qb)
                    sc = ps.tile([128, 1024], F32, tag="scw", name="scw",
                                 bufs=2)
                    for p in (0, 1):
                        pr = 64 * p
                        tmm(
                            sc[:, 512 * p + o : 512 * (p + 1)],
                            KT[hp][pr : pr + 64, 128 * kc : 128 * (kc + 1)],
                            QT[hp][pr : pr + 64,
                                   512 * qb + o : 512 * (qb + 1)],
                            start=True,
                            stop=True,
                        )
                    at = lp.tile([128, 1024], ADT, tag="at", name="at",
                                 bufs=4)
                    if o == 0:
                        nc.scalar.activation(at[:], sc[:], EXP, scale=0.125)
                    else:
                        # one strided activation covering both heads' live
                        # columns: [128, 2, 512-o] view with stride 512
                        nc.scalar.activation(
                            at[:].rearrange("p (two q) -> p two q",
                                            two=2)[:, :, o:512],
                            sc[:].rearrange("p (two q) -> p two q",
                                            two=2)[:, :, o:512],
                            EXP,
                            scale=0.125,
                        )
                    if kc >= 4 * qb:  # diagonal-crossing tile
                        for p in (0, 1):
                            blk = at[:, 512 * p + o : 512 * p + o + 128]
                            nc.vector.tensor_mul(blk, blk, tri[:])
                    at_tiles[i] = at

                def emit_back(i):
                    qb, hp, kc = iters[i]
                    nkc = 4 * qb + 4
                    o = max(0, 128 * kc - 512 * qb)
                    if kc == 0:
                        fv_tiles[(qb, hp)] = [
                            ps.tile([DK + 1, 512], F32, tag=f"fv{p}",
                                    name=f"fv{p}", bufs=2) for p in (0, 1)
                        ]
                    fv = fv_tiles[(qb, hp)]
                    at = at_tiles.pop(i)
                    for p in (0, 1):
                        h = 2 * hp + p
                        tmm(
                            fv[p][:, o:512],
                            VB[kc][:, h, :],
                            at[:, 512 * p + o : 512 * (p + 1)],
                            start=(kc == 0),
                            stop=(kc == nkc - 1),
                        )
                    if kc == nkc - 1:
                        # head-pair epilogue. Free the PSUM fv tiles FAST:
                        # copy denom rows + unnormalized CT immediately; the
                        # recip -> broadcast -> in-place scale chain then
                        # runs off the critical path.
                        drs = []
                        for p in (0, 1):
                            dr = lp.tile([1, 512], F32, tag=f"dr{p}",
                                         name=f"dr{p}", bufs=2)
                            nc.vector.tensor_copy(dr[:], fv[p][64:65, :])
                            drs.append(dr)
                        for p in (0, 1):
                            nc.vector.tensor_copy(
                                CT[qb][hp][64 * p : 64 * (p + 1), :],
                                fv[p][0:64, :],
                            )
                        for p in (0, 1):
                            nc.vector.reciprocal_approx_fast(drs[p][:],
                                                             drs[p][:])
                            rb = lp.tile([128, 512], F32, tag=f"rb{p}",
                                         name=f"rb{p}", bufs=2)
                            nc.gpsimd.partition_broadcast(rb[:], drs[p][:])
                            ct_sl = CT[qb][hp][64 * p : 64 * (p + 1), :]
                            nc.vector.tensor_mul(
                                ct_sl, ct_sl, rb[64 * p : 64 * (p + 1), :]
                            )
                        if hp == NJT - 1:
                            pending_p4.append((i + PLAG, qb))

                def emit_p4(qb):
                    for mtp in range(D // 256):
                        po = ps.tile([128, 1024], F32, tag="scw", name="scw",
                                     bufs=2)
                        for half in (0, 1):
                            mt = 2 * mtp + half
                            for c in range(NJT):
                                tmm(
                                    po[:, 512 * half : 512 * (half + 1)],
                                    WO[c][:, 128 * mt : 128 * (mt + 1)],
                                    CT[qb][c][:],
                                    start=(c == 0),
                                    stop=(c == NJT - 1),
                                )
                        for half in (0, 1):
                            mt = 2 * mtp + half
                            og = lp.tile([128, 512], F32, tag="ostg",
                                         name="ostg", bufs=6)
                            src = po[:, 512 * half : 512 * (half + 1)]
                            # scalar engine is busy with exp mid-kernel but
                            # idle at the tail; vector is the reverse.
                            if qb == NQB - 1:
                                nc.scalar.activation(og[:], src, COPY)
                            else:
                                nc.vector.tensor_copy(og[:], src)
                            nc.sync.dma_start(
                                OUT_d[128 * mt : 128 * (mt + 1),
                                      512 * qb : 512 * (qb + 1)],
                                og[:],
                            )

                for g in range(0, n_it + SKEW, 2):
                    for j in (g, g + 1):
                        if j < n_it:
                            emit_front(j)
                    for j in (g - SKEW, g + 1 - SKEW):
                        if 0 <= j < n_it:
                            emit_back(j)
                    while pending_p4 and pending_p4[0][0] <= g + 1 - SKEW:
                        emit_p4(pending_p4.pop(0)[1])
                while pending_p4:
                    emit_p4(pending_p4.pop(0)[1])
    nc.finalize()
    return nc


def _get_nc():
    if "nc" not in _NC_CACHE:
        _NC_CACHE["nc"] = build_nc()
    return _NC_CACHE["nc"]


def _make_in_maps(XKV, WQ, WK, WV, WO):
    bf = ml_dtypes.bfloat16
    in_maps = []
    for core in range(8):
        b, g = core // 2, core % 2
        sl = slice(512 * g, 512 * (g + 1))
        in_maps.append(
            {
                "XT": np.ascontiguousarray(XKV[b].T).astype(bf),
                "WQ": np.ascontiguousarray(WQ[:, sl]).astype(bf),
                "WK": np.ascontiguousarray(WK[:, sl]).astype(bf),
                "WV": np.ascontiguousarray(WV[:, sl]).astype(bf),
                "WO": np.ascontiguousarray(WO[sl, :]).astype(bf),
            }
        )
    return in_maps


def _combine(results, bO):
    out = np.empty((4, S, D), dtype=np.float32)
    for b in range(4):
        acc = results[2 * b]["OUTT"] + results[2 * b + 1]["OUTT"]
        out[b] = acc.T + bO[None, :]
    return out


def kernel(XKV, WQ, WK, WV, WO, bO):
    XKV = np.asarray(XKV, dtype=np.float32)
    nc = _get_nc()
    in_maps = _make_in_maps(XKV, np.asarray(WQ), np.asarray(WK), np.asarray(WV),
                            np.asarray(WO))
    res = run_bass_kernel_spmd(nc, in_maps, list(range(8)))
    return _combine(res.results, np.asarray(bO, dtype=np.float32))


# revision 17
# speedup vs baseline: 1.1895x; 1.1895x over previous
"""Multi-head causal attention (B=4, S=2048, D=1024, H=16, dk=dv=64) on 8
Trainium2 NeuronCores.

Sharding: batch (4) x head-group (2) -> 8 cores. Each core computes, for its
batch b and its 8 heads, the partial output (concat_g @ WO_g)^T in [D, S]
layout. Host sums the two head-group partials per batch, transposes, adds bO.

Per-core kernel (single SPMD Bass program, per-core data):
  P1: Q^T, K^T = (X @ WQg)^T, (X @ WKg)^T  as [512, 2048]  (j on partitions)
  P2: Vbar[s, h, 0:64] = (X @ WVg)[s, 512], Vbar[s, h, 64] = 1.0 (ones col)
  P3 (software-pipelined, skew 2, pair-grouped): per iteration
      (q-block 512, head pair, k-chunk 128):
      front(i): scoresT[k, q] for both heads of the pair into one wide
      [128,1024] PSUM tile (K=64 row-quadrants, adjacent so the PE co-issues
      them), one wide exp(0.125*scores) on the scalar engine, tri-mask of the
      diagonal-crossing cols on DVE.
      back(i-2): fv_p[0:65, q] += Vbar-chunk.T @ at_p  (row 64 = denom).
      Iterations are emitted in PAIRS ([sc(g) sc(g+1)] then [fv(g-2)
      fv(g-3)]) and every tensor matmul carries a nosync edge to the
      previous one, pinning the engine order: score and fv matmuls never
      alternate (PE tile-reconfig flush), and the exp latency is hidden by
      two iterations of skew.
      hp end: denom rows -> reciprocal_approx_fast -> gpsimd broadcast ->
      fused scale+cast into CT (bf16).
  P4 (lagged into the next q-block's scores): OUTT[m, q] = sum_c
      WOg-chunk.T @ CT-chunk, staged out via scalar/vector copies + DMA.

dtypes: inputs bf16; scores/fv accumulate f32 in PSUM; attn tiles bf16.
"""

import sys

sys.path.insert(0, "/opt/trn_rl_repo")

import numpy as np
import ml_dtypes

import concourse.bass as bass
from concourse import bacc
import concourse.tile as tile
from concourse import mybir
from concourse.bass_utils import run_bass_kernel_spmd

F32 = mybir.dt.float32
BF16 = mybir.dt.bfloat16
EXP = mybir.ActivationFunctionType.Exp
COPY = mybir.ActivationFunctionType.Copy
ADT = BF16   # dtype for Q/K/V/attn tiles (attention matmuls)

S = 2048          # sequence length
D = 1024          # model dim
HG = 8            # heads per core
DK = 64           # head dim
JG = HG * DK      # 512 = projected dim per core
CK = D // 128     # 8 contraction chunks for projections
NJT = JG // 128   # 4 j-tiles (head pairs)
NSB = S // 512    # 4 s-blocks of 512
NST = S // 128    # 16 s-tiles of 128
NQB = S // 512    # 4 q-blocks of 512

SKEW = 2          # fv trails scores by this many (qb,hp,kc) iterations
PLAG = 6          # out-projection trails the end of its q-block by this many

_NC_CACHE = {}


def build_nc(salt=""):
    nc = bacc.Bacc("TRN2", target_bir_lowering=False)

    XT_d = nc.declare_dram_parameter("XT", [D, S], BF16, isOutput=False)
    WQ_d = nc.declare_dram_parameter("WQ", [D, JG], BF16, isOutput=False)
    WK_d = nc.declare_dram_parameter("WK", [D, JG], BF16, isOutput=False)
    WV_d = nc.declare_dram_parameter("WV", [D, JG], BF16, isOutput=False)
    WO_d = nc.declare_dram_parameter("WO", [JG, D], BF16, isOutput=False)
    OUT_d = nc.declare_dram_parameter("OUTT", [D, S], F32, isOutput=True)

    # tril-inclusive mask in (k, q) orientation: mask[kr, qr] = 1 iff qr >= kr
    tri_np = (np.arange(128)[None, :] >= np.arange(128)[:, None]).astype(
        mybir.dt.np(ADT))
    TRI_d = nc.inline_tensor(tri_np, name="trimask" + salt)

    # Pin the tensor-engine order to emission order (see module docstring).
    import bass_rust as _br
    prev_mm = [None]

    def tmm(out, lhsT, rhs, **kw):
        mm = nc.tensor.matmul(out, lhsT, rhs, **kw)
        if prev_mm[0] is not None:
            deps = _br.InstructionNameOrderedSet()
            deps.add(prev_mm[0])
            mm.ins.add_nosync_dependencies_from(deps)
        prev_mm[0] = mm.ins.name
        return mm

    with tile.TileContext(nc) as tc:
        with (
            tc.tile_pool(name="persist", bufs=1) as pp,
            tc.tile_pool(name="psum", bufs=1, space="PSUM") as ps,
        ):
            tri = pp.tile([128, 128], ADT, tag="tri", name="tri")
            nc.sync.dma_start(tri[:], TRI_d[:])

            QT = [pp.tile([128, S], ADT, tag=f"qt{j}", name=f"qt{j}")
                  for j in range(NJT)]
            KT = [pp.tile([128, S], ADT, tag=f"kt{j}", name=f"kt{j}")
                  for j in range(NJT)]
            VB = [pp.tile([128, HG, DK + 1], ADT, tag=f"vb{s}", name=f"vb{s}")
                  for s in range(NST)]

            with tc.tile_pool(name="stage", bufs=1) as sp:
                XT = [sp.tile([128, S], BF16, tag=f"xt{c}", name=f"xt{c}")
                      for c in range(CK)]
                WQ = [sp.tile([128, JG], BF16, tag=f"wq{c}", name=f"wq{c}")
                      for c in range(CK)]
                WK = [sp.tile([128, JG], BF16, tag=f"wk{c}", name=f"wk{c}")
                      for c in range(CK)]
                WV = [sp.tile([128, JG], BF16, tag=f"wv{c}", name=f"wv{c}")
                      for c in range(CK)]
                # DMA in consumption order: P1-Q needs XT+WQ first.
                for c in range(CK):
                    nc.sync.dma_start(XT[c][:], XT_d[128 * c : 128 * (c + 1), :])
                    nc.sync.dma_start(WQ[c][:], WQ_d[128 * c : 128 * (c + 1), :])
                for c in range(CK):
                    nc.sync.dma_start(WK[c][:], WK_d[128 * c : 128 * (c + 1), :])
                for c in range(CK):
                    nc.sync.dma_start(WV[c][:], WV_d[128 * c : 128 * (c + 1), :])

                # ---- P1: Q^T and K^T projections ([j, s] layout) ----
                # Wide PSUM tiles: two 512-col s-blocks per tile, one wide
                # PSUM->SBUF cast per pair.
                for W, OT in ((WQ, QT), (WK, KT)):
                    for jt in range(NJT):
                        for sb in (0, 2):
                            pq = ps.tile([128, 1024], F32, tag="scw",
                                         name="scw", bufs=2)
                            for half in (0, 1):
                                for c in range(CK):
                                    tmm(
                                        pq[:, 512 * half : 512 * (half + 1)],
                                        W[c][:, 128 * jt : 128 * (jt + 1)],
                                        XT[c][:, 512 * (sb + half)
                                              : 512 * (sb + half + 1)],
                                        start=(c == 0),
                                        stop=(c == CK - 1),
                                    )
                            nc.vector.tensor_copy(
                                OT[jt][:, 512 * sb : 512 * (sb + 2)], pq[:]
                            )

                # ---- P2: V projection, seq-major, with ones column ----
                for st2 in range(0, NST, 2):
                    pv = ps.tile([128, 1024], F32, tag="scw", name="scw",
                                 bufs=2)
                    for half in (0, 1):
                        st = st2 + half
                        for c in range(CK):
                            tmm(
                                pv[:, 512 * half : 512 * (half + 1)],
                                XT[c][:, 128 * st : 128 * (st + 1)],
                                WV[c][:],
                                start=(c == 0),
                                stop=(c == CK - 1),
                            )
                    for half in (0, 1):
                        st = st2 + half
                        nc.vector.tensor_copy(
                            VB[st][:, :, 0:DK],
                            pv[:, 512 * half : 512 * (half + 1)].rearrange(
                                "p (h d) -> p h d", h=HG),
                        )
                        nc.gpsimd.memset(VB[st][:, :, DK : DK + 1], 1.0)

            with tc.tile_pool(name="late", bufs=1) as lp:
                WO = [lp.tile([128, D], BF16, tag=f"wo{c}", name=f"wo{c}")
                      for c in range(NJT)]
                for c in range(NJT):
                    nc.sync.dma_start(
                        WO[c][:], WO_d[128 * c : 128 * (c + 1), :]
                    )
                CT = [
                    [lp.tile([128, 512], BF16, tag=f"ct{qb}_{hp}",
                             name=f"ct{qb}_{hp}") for hp in range(NJT)]
                    for qb in range(NQB)
                ]

                # ---- P3: software-pipelined attention ----
                # Head-pair order interleaves q-blocks so the short qb=0
                # epilogue chains are covered by long neighbours, and qb3's
                # P4 lands mid-stream instead of in the tail.
                QB_ORDER = [3, 0, 2, 1]
                iters = [(qb, hp, kc)
                         for hp in range(NJT)
                         for qb in QB_ORDER
                         for kc in range(4 * qb + 4)]
                n_it = len(iters)
                at_tiles = {}   # i -> wide at tile
                fv_tiles = {}   # (qb, hp) -> (fv_p0, fv_p1)
                pending_p4 = [] # (emit_step, qb)
                done_hps = [0] * NQB
                last_p4_qb = QB_ORDER[-1]

                def emit_front(i):
                    qb, hp, kc = iters[i]
                    o = max(0, 128 * kc - 512 * qb)
                    sc = ps.tile([128, 1024], F32, tag="scw", name="scw",
                                 bufs=2)
                    for p in (0, 1):
                        pr = 64 * p
                        tmm(
                            sc[:, 512 * p + o : 512 * (p + 1)],
                            KT[hp][pr : pr + 64, 128 * kc : 128 * (kc + 1)],
                            QT[hp][pr : pr + 64,
                                   512 * qb + o : 512 * (qb + 1)],
                            start=True,
                            stop=True,
                        )
                    at = lp.tile([128, 1024], ADT, tag="at", name="at",
                                 bufs=4)
                    if o == 0:
                        nc.scalar.activation(at[:], sc[:], EXP, scale=0.125)
                    else:
                        for p in (0, 1):
                            nc.scalar.activation(
                                at[:, 512 * p + o : 512 * (p + 1)],
                                sc[:, 512 * p + o : 512 * (p + 1)],
                                EXP,
                                scale=0.125,
                            )
                    if kc >= 4 * qb:  # diagonal-crossing tile
                        for p in (0, 1):
                            blk = at[:, 512 * p + o : 512 * p + o + 128]
                            nc.vector.tensor_mul(blk, blk, tri[:])
                    at_tiles[i] = at

                def emit_back(i):
                    qb, hp, kc = iters[i]
                    nkc = 4 * qb + 4
                    o = max(0, 128 * kc - 512 * qb)
                    if kc == 0:
                        fv_tiles[(qb, hp)] = [
                            ps.tile([DK + 1, 512], F32, tag=f"fv{p}",
                                    name=f"fv{p}", bufs=2) for p in (0, 1)
                        ]
                    fv = fv_tiles[(qb, hp)]
                    at = at_tiles.pop(i)
                    for p in (0, 1):
                        h = 2 * hp + p
                        tmm(
                            fv[p][:, o:512],
                            VB[kc][:, h, :],
                            at[:, 512 * p + o : 512 * (p + 1)],
                            start=(kc == 0),
                            stop=(kc == nkc - 1),
                        )
                    if kc == nkc - 1:
                        # head-pair epilogue: denominators -> recip ->
                        # broadcast -> fused scale+cast into CT
                        for p in (0, 1):
                            dr = lp.tile([1, 512], F32, tag=f"dr{p}",
                                         name=f"dr{p}", bufs=2)
                            nc.vector.tensor_copy(dr[:], fv[p][64:65, :])
                            nc.vector.reciprocal_approx_fast(dr[:], dr[:])
                            rb = lp.tile([128, 512], F32, tag=f"rb{p}",
                                         name=f"rb{p}", bufs=2)
                            nc.gpsimd.partition_broadcast(rb[:], dr[:])
                            nc.vector.tensor_mul(
                                CT[qb][hp][64 * p : 64 * (p + 1), :],
                                fv[p][0:64, :],
                                rb[64 * p : 64 * (p + 1), :],
                            )
                        done_hps[qb] += 1
                        if done_hps[qb] == NJT:
                            pending_p4.append((i + PLAG, qb))

                def emit_p4(qb):
                    for mtp in range(D // 256):
                        po = ps.tile([128, 1024], F32, tag="scw", name="scw",
                                     bufs=2)
                        for half in (0, 1):
                            mt = 2 * mtp + half
                            for c in range(NJT):
                                tmm(
                                    po[:, 512 * half : 512 * (half + 1)],
                                    WO[c][:, 128 * mt : 128 * (mt + 1)],
                                    CT[qb][c][:],
                                    start=(c == 0),
                                    stop=(c == NJT - 1),
                                )
                        for half in (0, 1):
                            mt = 2 * mtp + half
                            og = lp.tile([128, 512], F32, tag="ostg",
                                         name="ostg", bufs=6)
                            src = po[:, 512 * half : 512 * (half + 1)]
                            # scalar engine is busy with exp mid-kernel but
                            # idle at the tail; vector is the reverse.
                            if qb == last_p4_qb:
                                nc.scalar.activation(og[:], src, COPY)
                            else:
                                nc.vector.tensor_copy(og[:], src)
                            nc.sync.dma_start(
                                OUT_d[128 * mt : 128 * (mt + 1),
                                      512 * qb : 512 * (qb + 1)],
                                og[:],
                            )

                for g in range(0, n_it + SKEW, 2):
                    for j in (g, g + 1):
                        if j < n_it:
                            emit_front(j)
                    for j in (g - SKEW, g + 1 - SKEW):
                        if 0 <= j < n_it:
                            emit_back(j)
                    while pending_p4 and pending_p4[0][0] <= g + 1 - SKEW:
                        emit_p4(pending_p4.pop(0)[1])
                while pending_p4:
                    emit_p4(pending_p4.pop(0)[1])
    nc.finalize()
    return nc


def _get_nc():
    if "nc" not in _NC_CACHE:
        _NC_CACHE["nc"] = build_nc()
    return _NC_CACHE["nc"]


def _make_in_maps(XKV, WQ, WK, WV, WO):
    bf = ml_dtypes.bfloat16
    in_maps = []
    for core in range(8):
        b, g = core // 2, core % 2
        sl = slice(512 * g, 512 * (g + 1))
        in_maps.append(
            {
                "XT": np.ascontiguousarray(XKV[b].T).astype(bf),
                "WQ": np.ascontiguousarray(WQ[:, sl]).astype(bf),
                "WK": np.ascontiguousarray(WK[:, sl]).astype(bf),
                "WV": np.ascontiguousarray(WV[:, sl]).astype(bf),
                "WO": np.ascontiguousarray(WO[sl, :]).astype(bf),
            }
        )
    return in_maps


def _combine(results, bO):
    out = np.empty((4, S, D), dtype=np.float32)
    for b in range(4):
        acc = results[2 * b]["OUTT"] + results[2 * b + 1]["OUTT"]
        out[b] = acc.T + bO[None, :]
    return out


def kernel(XKV, WQ, WK, WV, WO, bO):
    XKV = np.asarray(XKV, dtype=np.float32)
    nc = _get_nc()
    in_maps = _make_in_maps(XKV, np.asarray(WQ), np.asarray(WK), np.asarray(WV),
                            np.asarray(WO))
    res = run_bass_kernel_spmd(nc, in_maps, list(range(8)))
    return _combine(res.results, np.asarray(bO, dtype=np.float32))


# revision 22
# speedup vs baseline: 1.3624x; 1.1453x over previous
"""Multi-head causal attention (B=4, S=2048, D=1024, H=16, dk=dv=64) on 8
Trainium2 NeuronCores.

Sharding: batch (4) x head-group (2) -> 8 cores. Each core computes, for its
batch b and its 8 heads, the partial output (concat_g @ WO_g)^T in [D, S]
layout. Host sums the two head-group partials per batch, transposes, adds bO.

Per-core kernel (single SPMD Bass program, per-core data):
  P1: Q^T, K^T = (X @ WQg)^T, (X @ WKg)^T  as [512, 2048]  (j on partitions)
  P2: Vbar[s, h, 0:64] = (X @ WVg)[s, 512], Vbar[s, h, 64] = 1.0 (ones col)
  P3 (software-pipelined, skew 2, pair-grouped): per iteration
      (q-block 512, head pair, k-chunk 128):
      front(i): scoresT[k, q] for both heads of the pair into one wide
      [128,1024] PSUM tile (K=64 row-quadrants, adjacent so the PE co-issues
      them), one wide exp(0.125*scores) on the scalar engine, tri-mask of the
      diagonal-crossing cols on DVE.
      back(i-2): fv_p[0:65, q] += Vbar-chunk.T @ at_p  (row 64 = denom).
      Iterations are emitted in PAIRS ([sc(g) sc(g+1)] then [fv(g-2)
      fv(g-3)]) and every tensor matmul carries a nosync edge to the
      previous one, pinning the engine order: score and fv matmuls never
      alternate (PE tile-reconfig flush), and the exp latency is hidden by
      two iterations of skew.
      hp end: denom rows -> reciprocal_approx_fast -> gpsimd broadcast ->
      fused scale+cast into CT (bf16).
  P4 (lagged into the next q-block's scores): OUTT[m, q] = sum_c
      WOg-chunk.T @ CT-chunk, staged out via scalar/vector copies + DMA.

dtypes: inputs bf16; scores/fv accumulate f32 in PSUM; attn tiles bf16.
"""

import sys

sys.path.insert(0, "/opt/trn_rl_repo")

import numpy as np
import ml_dtypes

import concourse.bass as bass
from concourse import bacc
import concourse.tile as tile
from concourse import mybir
from concourse.bass_utils import run_bass_kernel_spmd

F32 = mybir.dt.float32
BF16 = mybir.dt.bfloat16
FP8 = mybir.dt.float8e4
DR = mybir.MatmulPerfMode.DoubleRow
EXP = mybir.ActivationFunctionType.Exp
COPY = mybir.ActivationFunctionType.Copy
ADT = BF16   # dtype for Q/K/V/attn tiles (attention matmuls)

# The Q/K projections run in fp8e4 DoubleRow mode. WQ/WK are pre-scaled by
# QK_WSCALE on the host (lifts the tiny uniform(-1/32,1/32) weights out of
# the fp8 subnormal range); the exp() then folds 1/QK_WSCALE^2 into its
# scale: exp(scores/8) = exp(scaled_scores * 0.125 / QK_WSCALE^2).
QK_WSCALE = 16.0
EXP_SCALE = 0.125 / (QK_WSCALE * QK_WSCALE)

S = 2048          # sequence length
D = 1024          # model dim
HG = 8            # heads per core
DK = 64           # head dim
JG = HG * DK      # 512 = projected dim per core
CK = D // 128     # 8 contraction chunks for projections
NJT = JG // 128   # 4 j-tiles (head pairs)
NSB = S // 512    # 4 s-blocks of 512
NST = S // 128    # 16 s-tiles of 128
NQB = S // 512    # 4 q-blocks of 512

SKEW = 2          # fv trails scores by this many (qb,hp,kc) iterations
PLAG = 6          # out-projection trails the end of its q-block by this many

_NC_CACHE = {}


def build_nc(salt=""):
    nc = bacc.Bacc("TRN2", target_bir_lowering=False)

    XT_d = nc.declare_dram_parameter("XT", [D, S], BF16, isOutput=False)
    X8_d = nc.declare_dram_parameter("X8", [D // 256, 128, 2, S], FP8,
                                     isOutput=False)
    WQ8_d = nc.declare_dram_parameter("WQ8", [D // 256, 128, 2, JG], FP8,
                                      isOutput=False)
    WK8_d = nc.declare_dram_parameter("WK8", [D // 256, 128, 2, JG], FP8,
                                      isOutput=False)
    WV_d = nc.declare_dram_parameter("WV", [D, JG], BF16, isOutput=False)
    WO_d = nc.declare_dram_parameter("WO", [JG, D], BF16, isOutput=False)
    OUT_d = nc.declare_dram_parameter("OUTT", [D, S], BF16, isOutput=True)

    # tril-inclusive mask in (k, q) orientation: mask[kr, qr] = 1 iff qr >= kr
    tri_np = (np.arange(128)[None, :] >= np.arange(128)[:, None]).astype(
        mybir.dt.np(ADT))
    TRI_d = nc.inline_tensor(tri_np, name="trimask" + salt)

    # Pin the tensor-engine order to emission order (see module docstring).
    import bass_rust as _br
    prev_mm = [None]

    def tmm(out, lhsT, rhs, **kw):
        mm = nc.tensor.matmul(out, lhsT, rhs, **kw)
        if prev_mm[0] is not None:
            deps = _br.InstructionNameOrderedSet()
            deps.add(prev_mm[0])
            mm.ins.add_nosync_dependencies_from(deps)
        prev_mm[0] = mm.ins.name
        return mm

    with tile.TileContext(nc) as tc:
        with (
            tc.tile_pool(name="persist", bufs=1) as pp,
            tc.tile_pool(name="psum", bufs=1, space="PSUM") as ps,
        ):
            tri = pp.tile([128, 128], ADT, tag="tri", name="tri")
            nc.sync.dma_start(tri[:], TRI_d[:])

            QT = [pp.tile([128, S], ADT, tag=f"qt{j}", name=f"qt{j}")
                  for j in range(NJT)]
            KT = [pp.tile([128, S], ADT, tag=f"kt{j}", name=f"kt{j}")
                  for j in range(NJT)]
            VB = [pp.tile([128, HG, DK + 1], ADT, tag=f"vb{s}", name=f"vb{s}")
                  for s in range(NST)]

            with tc.tile_pool(name="stage", bufs=1) as sp:
                C2 = D // 256   # 4 DoubleRow contraction chunks
                X8 = [sp.tile([128, 2, S], FP8, tag=f"x8{c}", name=f"x8{c}")
                      for c in range(C2)]
                WQ8 = [sp.tile([128, 2, JG], FP8, tag=f"wq{c}", name=f"wq{c}")
                       for c in range(C2)]
                WK8 = [sp.tile([128, 2, JG], FP8, tag=f"wk{c}", name=f"wk{c}")
                       for c in range(C2)]
                XT = [sp.tile([128, S], BF16, tag=f"xt{c}", name=f"xt{c}")
                      for c in range(CK)]
                WV = [sp.tile([128, JG], BF16, tag=f"wv{c}", name=f"wv{c}")
                      for c in range(CK)]
                # DMA in consumption order: P1-Q needs X8+WQ8 first.
                for c in range(C2):
                    nc.sync.dma_start(X8[c][:], X8_d[c])
                    nc.sync.dma_start(WQ8[c][:], WQ8_d[c])
                for c in range(C2):
                    nc.sync.dma_start(WK8[c][:], WK8_d[c])
                for c in range(CK):
                    nc.sync.dma_start(XT[c][:], XT_d[128 * c : 128 * (c + 1), :])
                    nc.sync.dma_start(WV[c][:], WV_d[128 * c : 128 * (c + 1), :])

                # ---- P1: Q^T and K^T projections ([j, s] layout) ----
                # fp8e4 DoubleRow: each matmul contracts 256 model dims.
                for W8, OT in ((WQ8, QT), (WK8, KT)):
                    for jt in range(NJT):
                        for sb in (0, 2):
                            pq = ps.tile([128, 1024], F32, tag="scw",
                                         name="scw", bufs=2)
                            for half in (0, 1):
                                for c in range(C2):
                                    tmm(
                                        pq[:, 512 * half : 512 * (half + 1)],
                                        W8[c][:, :,
                                              128 * jt : 128 * (jt + 1)],
                                        X8[c][:, :, 512 * (sb + half)
                                              : 512 * (sb + half + 1)],
                                        start=(c == 0),
                                        stop=(c == C2 - 1),
                                        perf_mode=DR,
                                    )
                            nc.vector.tensor_copy(
                                OT[jt][:, 512 * sb : 512 * (sb + 2)], pq[:]
                            )

                # ---- P2: V projection, seq-major, with ones column ----
                for st2 in range(0, NST, 2):
                    pv = ps.tile([128, 1024], F32, tag="scw", name="scw",
                                 bufs=2)
                    for half in (0, 1):
                        st = st2 + half
                        for c in range(CK):
                            tmm(
                                pv[:, 512 * half : 512 * (half + 1)],
                                XT[c][:, 128 * st : 128 * (st + 1)],
                                WV[c][:],
                                start=(c == 0),
                                stop=(c == CK - 1),
                            )
                    for half in (0, 1):
                        st = st2 + half
                        nc.vector.tensor_copy(
                            VB[st][:, :, 0:DK],
                            pv[:, 512 * half : 512 * (half + 1)].rearrange(
                                "p (h d) -> p h d", h=HG),
                        )
                        nc.gpsimd.memset(VB[st][:, :, DK : DK + 1], 1.0)

            with tc.tile_pool(name="late", bufs=1) as lp:
                WO = [lp.tile([128, D], BF16, tag=f"wo{c}", name=f"wo{c}")
                      for c in range(NJT)]
                for c in range(NJT):
                    nc.sync.dma_start(
                        WO[c][:], WO_d[128 * c : 128 * (c + 1), :]
                    )
                CT = [
                    [lp.tile([128, 512], BF16, tag=f"ct{qb}_{hp}",
                             name=f"ct{qb}_{hp}") for hp in range(NJT)]
                    for qb in range(NQB)
                ]

                # ---- P3: software-pipelined attention ----
                # Head-pair order interleaves q-blocks so the short qb=0
                # epilogue chains are covered by long neighbours, and qb3's
                # P4 lands mid-stream instead of in the tail.
                QB_ORDER = [3, 0, 2, 1]
                iters = [(qb, hp, kc)
                         for qb in QB_ORDER
                         for hp in range(NJT)
                         for kc in range(4 * qb + 4)]
                n_it = len(iters)
                at_tiles = {}   # i -> wide at tile
                fv_tiles = {}   # (qb, hp) -> (fv_p0, fv_p1)
                pending_p4 = [] # (emit_step, qb)
                done_hps = [0] * NQB
                last_p4_qb = QB_ORDER[-1]

                def emit_front(i):
                    qb, hp, kc = iters[i]
                    o = max(0, 128 * kc - 512 * qb)
                    sc = ps.tile([128, 1024], F32, tag="scw", name="scw",
                                 bufs=2)
                    for p in (0, 1):
                        pr = 64 * p
                        tmm(
                            sc[:, 512 * p + o : 512 * (p + 1)],
                            KT[hp][pr : pr + 64, 128 * kc : 128 * (kc + 1)],
                            QT[hp][pr : pr + 64,
                                   512 * qb + o : 512 * (qb + 1)],
                            start=True,
                            stop=True,
                        )
                    at = lp.tile([128, 1024], ADT, tag="at", name="at",
                                 bufs=4)
                    if o == 0:
                        nc.scalar.activation(at[:], sc[:], EXP, scale=EXP_SCALE)
                    else:
                        for p in (0, 1):
                            nc.scalar.activation(
                                at[:, 512 * p + o : 512 * (p + 1)],
                                sc[:, 512 * p + o : 512 * (p + 1)],
                                EXP,
                                scale=EXP_SCALE,
                            )
                    if kc >= 4 * qb:  # diagonal-crossing tile
                        for p in (0, 1):
                            blk = at[:, 512 * p + o : 512 * p + o + 128]
                            nc.vector.tensor_mul(blk, blk, tri[:])
                    at_tiles[i] = at

                def emit_back(i):
                    qb, hp, kc = iters[i]
                    nkc = 4 * qb + 4
                    o = max(0, 128 * kc - 512 * qb)
                    if kc == 0:
                        fv_tiles[(qb, hp)] = [
                            ps.tile([DK + 1, 512], F32, tag=f"fv{p}",
                                    name=f"fv{p}", bufs=2) for p in (0, 1)
                        ]
                    fv = fv_tiles[(qb, hp)]
                    at = at_tiles.pop(i)
                    for p in (0, 1):
                        h = 2 * hp + p
                        tmm(
                            fv[p][:, o:512],
                            VB[kc][:, h, :],
                            at[:, 512 * p + o : 512 * (p + 1)],
                            start=(kc == 0),
                            stop=(kc == nkc - 1),
                        )
                    if kc == nkc - 1:
                        # head-pair epilogue: denominators -> recip ->
                        # broadcast -> fused scale+cast into CT
                        for p in (0, 1):
                            dr = lp.tile([1, 512], F32, tag=f"dr{p}",
                                         name=f"dr{p}", bufs=2)
                            nc.vector.tensor_copy(dr[:], fv[p][64:65, :])
                            nc.vector.reciprocal_approx_fast(dr[:], dr[:])
                            rb = lp.tile([128, 512], F32, tag=f"rb{p}",
                                         name=f"rb{p}", bufs=2)
                            nc.gpsimd.partition_broadcast(rb[:], dr[:])
                            nc.vector.tensor_mul(
                                CT[qb][hp][64 * p : 64 * (p + 1), :],
                                fv[p][0:64, :],
                                rb[64 * p : 64 * (p + 1), :],
                            )
                        done_hps[qb] += 1
                        if done_hps[qb] == NJT:
                            pending_p4.append((i + PLAG, qb))

                def emit_p4(qb):
                    for mtp in range(D // 256):
                        po = ps.tile([128, 1024], F32, tag="scw", name="scw",
                                     bufs=2)
                        for half in (0, 1):
                            mt = 2 * mtp + half
                            for c in range(NJT):
                                tmm(
                                    po[:, 512 * half : 512 * (half + 1)],
                                    WO[c][:, 128 * mt : 128 * (mt + 1)],
                                    CT[qb][c][:],
                                    start=(c == 0),
                                    stop=(c == NJT - 1),
                                )
                        for half in (0, 1):
                            mt = 2 * mtp + half
                            og = lp.tile([128, 512], BF16, tag="ostg",
                                         name="ostg", bufs=6)
                            src = po[:, 512 * half : 512 * (half + 1)]
                            # scalar engine is busy with exp mid-kernel but
                            # idle at the tail; vector is the reverse.
                            if qb == last_p4_qb:
                                nc.scalar.activation(og[:], src, COPY)
                            else:
                                nc.vector.tensor_copy(og[:], src)
                            nc.sync.dma_start(
                                OUT_d[128 * mt : 128 * (mt + 1),
                                      512 * qb : 512 * (qb + 1)],
                                og[:],
                            )

                for g in range(0, n_it + SKEW, 2):
                    for j in (g, g + 1):
                        if j < n_it:
                            emit_front(j)
                    for j in (g - SKEW, g + 1 - SKEW):
                        if 0 <= j < n_it:
                            emit_back(j)
                    while pending_p4 and pending_p4[0][0] <= g + 1 - SKEW:
                        emit_p4(pending_p4.pop(0)[1])
                while pending_p4:
                    emit_p4(pending_p4.pop(0)[1])
    nc.finalize()
    return nc


def _get_nc():
    if "nc" not in _NC_CACHE:
        _NC_CACHE["nc"] = build_nc()
    return _NC_CACHE["nc"]


def _dr_pack(arr, ncols):
    """[1024, ncols] -> [4, 128, 2, ncols] fp8e4 DoubleRow layout."""
    f8 = ml_dtypes.float8_e4m3
    return np.ascontiguousarray(
        arr.reshape(4, 2, 128, ncols).transpose(0, 2, 1, 3)).astype(f8)


def _make_in_maps(XKV, WQ, WK, WV, WO):
    bf = ml_dtypes.bfloat16
    in_maps = []
    xt_b = [np.ascontiguousarray(XKV[b].T) for b in range(4)]
    x8_b = [_dr_pack(x, S) for x in xt_b]
    for core in range(8):
        b, g = core // 2, core % 2
        sl = slice(512 * g, 512 * (g + 1))
        in_maps.append(
            {
                "XT": xt_b[b].astype(bf),
                "X8": x8_b[b],
                "WQ8": _dr_pack(QK_WSCALE * np.asarray(WQ[:, sl],
                                                       dtype=np.float32), JG),
                "WK8": _dr_pack(QK_WSCALE * np.asarray(WK[:, sl],
                                                       dtype=np.float32), JG),
                "WV": np.ascontiguousarray(WV[:, sl]).astype(bf),
                "WO": np.ascontiguousarray(WO[sl, :]).astype(bf),
            }
        )
    return in_maps


def _combine(results, bO):
    out = np.empty((4, S, D), dtype=np.float32)
    for b in range(4):
        acc = (results[2 * b]["OUTT"].astype(np.float32)
               + results[2 * b + 1]["OUTT"].astype(np.float32))
        out[b] = acc.T + bO[None, :]
    return out


def kernel(XKV, WQ, WK, WV, WO, bO):
    XKV = np.asarray(XKV, dtype=np.float32)
    nc = _get_nc()
    in_maps = _make_in_maps(XKV, np.asarray(WQ), np.asarray(WK), np.asarray(WV),
                            np.asarray(WO))
    res = run_bass_kernel_spmd(nc, in_maps, list(range(8)))
    return _combine(res.results, np.asarray(bO, dtype=np.float32))


# revision 23
# speedup vs baseline: 1.4222x; 1.0439x over previous
"""Multi-head causal attention (B=4, S=2048, D=1024, H=16, dk=dv=64) on 8
Trainium2 NeuronCores.

Sharding: batch (4) x head-group (2) -> 8 cores. Each core computes, for its
batch b and its 8 heads, the partial output (concat_g @ WO_g)^T in [D, S]
layout. Host sums the two head-group partials per batch, transposes, adds bO.

Per-core kernel (single SPMD Bass program, per-core data):
  P1: Q^T, K^T = (X @ WQg)^T, (X @ WKg)^T  as [512, 2048]  (j on partitions)
  P2: Vbar[s, h, 0:64] = (X @ WVg)[s, 512], Vbar[s, h, 64] = 1.0 (ones col)
  P3 (software-pipelined, skew 2, pair-grouped): per iteration
      (q-block 512, head pair, k-chunk 128):
      front(i): scoresT[k, q] for both heads of the pair into one wide
      [128,1024] PSUM tile (K=64 row-quadrants, adjacent so the PE co-issues
      them), one wide exp(0.125*scores) on the scalar engine, tri-mask of the
      diagonal-crossing cols on DVE.
      back(i-2): fv_p[0:65, q] += Vbar-chunk.T @ at_p  (row 64 = denom).
      Iterations are emitted in PAIRS ([sc(g) sc(g+1)] then [fv(g-2)
      fv(g-3)]) and every tensor matmul carries a nosync edge to the
      previous one, pinning the engine order: score and fv matmuls never
      alternate (PE tile-reconfig flush), and the exp latency is hidden by
      two iterations of skew.
      hp end: denom rows -> reciprocal_approx_fast -> gpsimd broadcast ->
      fused scale+cast into CT (bf16).
  P4 (lagged into the next q-block's scores): OUTT[m, q] = sum_c
      WOg-chunk.T @ CT-chunk, staged out via scalar/vector copies + DMA.

dtypes: inputs bf16; scores/fv accumulate f32 in PSUM; attn tiles bf16.
"""

import sys

sys.path.insert(0, "/opt/trn_rl_repo")

import numpy as np
import ml_dtypes

import concourse.bass as bass
from concourse import bacc
import concourse.tile as tile
from concourse import mybir
from concourse.bass_utils import run_bass_kernel_spmd

F32 = mybir.dt.float32
BF16 = mybir.dt.bfloat16
FP8 = mybir.dt.float8e4
DR = mybir.MatmulPerfMode.DoubleRow
EXP = mybir.ActivationFunctionType.Exp
COPY = mybir.ActivationFunctionType.Copy
ADT = BF16   # dtype for Q/K/V/attn tiles (attention matmuls)

# The Q/K projections run in fp8e4 DoubleRow mode. WQ/WK are pre-scaled by
# QK_WSCALE on the host (lifts the tiny uniform(-1/32,1/32) weights out of
# the fp8 subnormal range); the exp() then folds 1/QK_WSCALE^2 into its
# scale: exp(scores/8) = exp(scaled_scores * 0.125 / QK_WSCALE^2).
QK_WSCALE = 16.0
EXP_SCALE = 0.125 / (QK_WSCALE * QK_WSCALE)

S = 2048          # sequence length
D = 1024          # model dim
HG = 8            # heads per core
DK = 64           # head dim
JG = HG * DK      # 512 = projected dim per core
CK = D // 128     # 8 contraction chunks for projections
NJT = JG // 128   # 4 j-tiles (head pairs)
NSB = S // 512    # 4 s-blocks of 512
NST = S // 128    # 16 s-tiles of 128
NQB = S // 512    # 4 q-blocks of 512

SKEW = 2          # fv trails scores by this many (qb,hp,kc) iterations
PLAG = 6          # out-projection trails the end of its q-block by this many

_NC_CACHE = {}


def build_nc(salt=""):
    nc = bacc.Bacc("TRN2", target_bir_lowering=False)

    XT_d = nc.declare_dram_parameter("XT", [D, S], BF16, isOutput=False)
    X8_d = nc.declare_dram_parameter("X8", [D // 256, 128, 2, S], FP8,
                                     isOutput=False)
    WQ8_d = nc.declare_dram_parameter("WQ8", [D // 256, 128, 2, JG], FP8,
                                      isOutput=False)
    WK8_d = nc.declare_dram_parameter("WK8", [D // 256, 128, 2, JG], FP8,
                                      isOutput=False)
    WV_d = nc.declare_dram_parameter("WV", [D, JG], BF16, isOutput=False)
    WO_d = nc.declare_dram_parameter("WO", [JG, D], BF16, isOutput=False)
    OUT_d = nc.declare_dram_parameter("OUTT", [D, S], BF16, isOutput=True)

    # tril-inclusive mask in (k, q) orientation: mask[kr, qr] = 1 iff qr >= kr
    tri_np = (np.arange(128)[None, :] >= np.arange(128)[:, None]).astype(
        mybir.dt.np(ADT))
    TRI_d = nc.inline_tensor(tri_np, name="trimask" + salt)

    # Pin the tensor-engine order to emission order (see module docstring).
    import bass_rust as _br
    prev_mm = [None]

    def tmm(out, lhsT, rhs, **kw):
        mm = nc.tensor.matmul(out, lhsT, rhs, **kw)
        if prev_mm[0] is not None:
            deps = _br.InstructionNameOrderedSet()
            deps.add(prev_mm[0])
            mm.ins.add_nosync_dependencies_from(deps)
        prev_mm[0] = mm.ins.name
        return mm

    with tile.TileContext(nc) as tc:
        with (
            tc.tile_pool(name="persist", bufs=1) as pp,
            tc.tile_pool(name="psum", bufs=1, space="PSUM") as ps,
        ):
            tri = pp.tile([128, 128], ADT, tag="tri", name="tri")
            nc.sync.dma_start(tri[:], TRI_d[:])

            QT = [pp.tile([128, S], ADT, tag=f"qt{j}", name=f"qt{j}")
                  for j in range(NJT)]
            KT = [pp.tile([128, S], ADT, tag=f"kt{j}", name=f"kt{j}")
                  for j in range(NJT)]
            VB = [pp.tile([128, HG, DK + 1], ADT, tag=f"vb{s}", name=f"vb{s}")
                  for s in range(NST)]

            with tc.tile_pool(name="stage", bufs=1) as sp:
                C2 = D // 256   # 4 DoubleRow contraction chunks
                X8 = [sp.tile([128, 2, S], FP8, tag=f"x8{c}", name=f"x8{c}")
                      for c in range(C2)]
                WQ8 = [sp.tile([128, 2, JG], FP8, tag=f"wq{c}", name=f"wq{c}")
                       for c in range(C2)]
                WK8 = [sp.tile([128, 2, JG], FP8, tag=f"wk{c}", name=f"wk{c}")
                       for c in range(C2)]
                XT = [sp.tile([128, S], BF16, tag=f"xt{c}", name=f"xt{c}")
                      for c in range(CK)]
                WV = [sp.tile([128, JG], BF16, tag=f"wv{c}", name=f"wv{c}")
                      for c in range(CK)]
                # DMA in consumption order: P1-Q consumes s-cols 0:1024
                # of every chunk first (sb-pair 0), so ship those halves
                # before the 1024:2048 halves.
                for c in range(C2):
                    nc.sync.dma_start(X8[c][:, :, 0:1024],
                                      X8_d[c][:, :, 0:1024])
                    nc.sync.dma_start(WQ8[c][:], WQ8_d[c])
                for c in range(C2):
                    nc.sync.dma_start(X8[c][:, :, 1024:2048],
                                      X8_d[c][:, :, 1024:2048])
                for c in range(C2):
                    nc.sync.dma_start(WK8[c][:], WK8_d[c])
                for c in range(CK):
                    nc.sync.dma_start(XT[c][:], XT_d[128 * c : 128 * (c + 1), :])
                    nc.sync.dma_start(WV[c][:], WV_d[128 * c : 128 * (c + 1), :])

                # ---- P1: Q^T and K^T projections ([j, s] layout) ----
                # fp8e4 DoubleRow: each matmul contracts 256 model dims.
                for W8, OT in ((WQ8, QT), (WK8, KT)):
                    for sb in (0, 2):
                        for jt in range(NJT):
                            pq = ps.tile([128, 1024], F32, tag="scw",
                                         name="scw", bufs=2)
                            for half in (0, 1):
                                for c in range(C2):
                                    tmm(
                                        pq[:, 512 * half : 512 * (half + 1)],
                                        W8[c][:, :,
                                              128 * jt : 128 * (jt + 1)],
                                        X8[c][:, :, 512 * (sb + half)
                                              : 512 * (sb + half + 1)],
                                        start=(c == 0),
                                        stop=(c == C2 - 1),
                                        perf_mode=DR,
                                    )
                            nc.vector.tensor_copy(
                                OT[jt][:, 512 * sb : 512 * (sb + 2)], pq[:]
                            )

                # ---- P2: V projection, seq-major, with ones column ----
                for st2 in range(0, NST, 2):
                    pv = ps.tile([128, 1024], F32, tag="scw", name="scw",
                                 bufs=2)
                    for half in (0, 1):
                        st = st2 + half
                        for c in range(CK):
                            tmm(
                                pv[:, 512 * half : 512 * (half + 1)],
                                XT[c][:, 128 * st : 128 * (st + 1)],
                                WV[c][:],
                                start=(c == 0),
                                stop=(c == CK - 1),
                            )
                    for half in (0, 1):
                        st = st2 + half
                        nc.vector.tensor_copy(
                            VB[st][:, :, 0:DK],
                            pv[:, 512 * half : 512 * (half + 1)].rearrange(
                                "p (h d) -> p h d", h=HG),
                        )
                        nc.gpsimd.memset(VB[st][:, :, DK : DK + 1], 1.0)

            with tc.tile_pool(name="late", bufs=1) as lp:
                WO = [lp.tile([128, D], BF16, tag=f"wo{c}", name=f"wo{c}")
                      for c in range(NJT)]
                for c in range(NJT):
                    nc.sync.dma_start(
                        WO[c][:], WO_d[128 * c : 128 * (c + 1), :]
                    )
                CT = [
                    [lp.tile([128, 512], BF16, tag=f"ct{qb}_{hp}",
                             name=f"ct{qb}_{hp}") for hp in range(NJT)]
                    for qb in range(NQB)
                ]

                # ---- P3: software-pipelined attention ----
                # Head-pair order interleaves q-blocks so the short qb=0
                # epilogue chains are covered by long neighbours, and qb3's
                # P4 lands mid-stream instead of in the tail.
                QB_ORDER = [3, 0, 2, 1]
                iters = [(qb, hp, kc)
                         for qb in QB_ORDER
                         for hp in range(NJT)
                         for kc in range(4 * qb + 4)]
                n_it = len(iters)
                at_tiles = {}   # i -> wide at tile
                fv_tiles = {}   # (qb, hp) -> (fv_p0, fv_p1)
                pending_p4 = [] # (emit_step, qb)
                done_hps = [0] * NQB
                last_p4_qb = QB_ORDER[-1]

                def emit_front(i):
                    qb, hp, kc = iters[i]
                    o = max(0, 128 * kc - 512 * qb)
                    # head0 live cols [o, 512), head1 live cols
                    # [512, 1024-o): contiguous union -> single exp instr.
                    sc = ps.tile([128, 1024], F32, tag="scw", name="scw",
                                 bufs=2)
                    for p in (0, 1):
                        pr = 64 * p
                        lo = o if p == 0 else 512
                        tmm(
                            sc[:, lo : lo + 512 - o],
                            KT[hp][pr : pr + 64, 128 * kc : 128 * (kc + 1)],
                            QT[hp][pr : pr + 64,
                                   512 * qb + o : 512 * (qb + 1)],
                            start=True,
                            stop=True,
                        )
                    at = lp.tile([128, 1024], ADT, tag="at", name="at",
                                 bufs=4)
                    nc.scalar.activation(at[:, o : 1024 - o],
                                         sc[:, o : 1024 - o], EXP,
                                         scale=EXP_SCALE)
                    if kc >= 4 * qb:  # diagonal-crossing tile
                        for p in (0, 1):
                            lo = o if p == 0 else 512
                            blk = at[:, lo : lo + 128]
                            nc.vector.tensor_mul(blk, blk, tri[:])
                    at_tiles[i] = at

                def emit_back(i):
                    qb, hp, kc = iters[i]
                    nkc = 4 * qb + 4
                    o = max(0, 128 * kc - 512 * qb)
                    if kc == 0:
                        fv_tiles[(qb, hp)] = [
                            ps.tile([DK + 1, 512], F32, tag=f"fv{p}",
                                    name=f"fv{p}", bufs=2) for p in (0, 1)
                        ]
                    fv = fv_tiles[(qb, hp)]
                    at = at_tiles.pop(i)
                    for p in (0, 1):
                        h = 2 * hp + p
                        lo = o if p == 0 else 512
                        tmm(
                            fv[p][:, o:512],
                            VB[kc][:, h, :],
                            at[:, lo : lo + 512 - o],
                            start=(kc == 0),
                            stop=(kc == nkc - 1),
                        )
                    if kc == nkc - 1:
                        # head-pair epilogue: denominators -> recip ->
                        # broadcast -> fused scale+cast into CT
                        for p in (0, 1):
                            dr = lp.tile([1, 512], F32, tag=f"dr{p}",
                                         name=f"dr{p}", bufs=2)
                            nc.vector.tensor_copy(dr[:], fv[p][64:65, :])
                            nc.vector.reciprocal_approx_fast(dr[:], dr[:])
                            rb = lp.tile([128, 512], F32, tag=f"rb{p}",
                                         name=f"rb{p}", bufs=2)
                            nc.gpsimd.partition_broadcast(rb[:], dr[:])
                            nc.vector.tensor_mul(
                                CT[qb][hp][64 * p : 64 * (p + 1), :],
                                fv[p][0:64, :],
                                rb[64 * p : 64 * (p + 1), :],
                            )
                        done_hps[qb] += 1
                        if done_hps[qb] == NJT:
                            for mtp in range(D // 256):
                                pending_p4.append((i + PLAG + 2 * mtp, qb,
                                                   mtp))

                def emit_p4_group(qb, mtp):
                    po = ps.tile([128, 1024], F32, tag="scw", name="scw",
                                 bufs=2)
                    for half in (0, 1):
                        mt = 2 * mtp + half
                        for c in range(NJT):
                            tmm(
                                po[:, 512 * half : 512 * (half + 1)],
                                WO[c][:, 128 * mt : 128 * (mt + 1)],
                                CT[qb][c][:],
                                start=(c == 0),
                                stop=(c == NJT - 1),
                            )
                    for half in (0, 1):
                        mt = 2 * mtp + half
                        og = lp.tile([128, 512], BF16, tag="ostg",
                                     name="ostg", bufs=6)
                        osrc = po[:, 512 * half : 512 * (half + 1)]
                        # split the PSUM-evacuation casts across scalar and
                        # vector so neither queue gates the scw rotation;
                        # the tail q-block goes all-scalar (exp is done).
                        if qb == last_p4_qb or half == 1:
                            nc.scalar.activation(og[:], osrc, COPY)
                        else:
                            nc.vector.tensor_copy(og[:], osrc)
                        nc.sync.dma_start(
                            OUT_d[128 * mt : 128 * (mt + 1),
                                  512 * qb : 512 * (qb + 1)],
                            og[:],
                        )

                for g in range(0, n_it + SKEW, 2):
                    for j in (g, g + 1):
                        if j < n_it:
                            emit_front(j)
                    for j in (g - SKEW, g + 1 - SKEW):
                        if 0 <= j < n_it:
                            emit_back(j)
                    while pending_p4 and pending_p4[0][0] <= g + 1 - SKEW:
                        _, pqb, pmtp = pending_p4.pop(0)
                        emit_p4_group(pqb, pmtp)
                while pending_p4:
                    _, pqb, pmtp = pending_p4.pop(0)
                    emit_p4_group(pqb, pmtp)
    nc.finalize()
    return nc


def _get_nc():
    if "nc" not in _NC_CACHE:
        _NC_CACHE["nc"] = build_nc()
    return _NC_CACHE["nc"]


def _dr_pack(arr, ncols):
    """[1024, ncols] -> [4, 128, 2, ncols] fp8e4 DoubleRow layout."""
    f8 = ml_dtypes.float8_e4m3
    return np.ascontiguousarray(
        arr.reshape(4, 2, 128, ncols).transpose(0, 2, 1, 3)).astype(f8)


def _make_in_maps(XKV, WQ, WK, WV, WO):
    bf = ml_dtypes.bfloat16
    in_maps = []
    xt_b = [np.ascontiguousarray(XKV[b].T) for b in range(4)]
    x8_b = [_dr_pack(x, S) for x in xt_b]
    for core in range(8):
        b, g = core // 2, core % 2
        sl = slice(512 * g, 512 * (g + 1))
        in_maps.append(
            {
                "XT": xt_b[b].astype(bf),
                "X8": x8_b[b],
                "WQ8": _dr_pack(QK_WSCALE * np.asarray(WQ[:, sl],
                                                       dtype=np.float32), JG),
                "WK8": _dr_pack(QK_WSCALE * np.asarray(WK[:, sl],
                                                       dtype=np.float32), JG),
                "WV": np.ascontiguousarray(WV[:, sl]).astype(bf),
                "WO": np.ascontiguousarray(WO[sl, :]).astype(bf),
            }
        )
    return in_maps


def _combine(results, bO):
    out = np.empty((4, S, D), dtype=np.float32)
    for b in range(4):
        acc = (results[2 * b]["OUTT"].astype(np.float32)
               + results[2 * b + 1]["OUTT"].astype(np.float32))
        out[b] = acc.T + bO[None, :]
    return out


def kernel(XKV, WQ, WK, WV, WO, bO):
    XKV = np.asarray(XKV, dtype=np.float32)
    nc = _get_nc()
    in_maps = _make_in_maps(XKV, np.asarray(WQ), np.asarray(WK), np.asarray(WV),
                            np.asarray(WO))
    res = run_bass_kernel_spmd(nc, in_maps, list(range(8)))
    return _combine(res.results, np.asarray(bO, dtype=np.float32))
